# revision 7
# baseline (speedup 1.0000x reference)
"""Trainium2 Bass kernel v2 for nn_EntityEncoder (adapters + BiLSTM + proj).

Sharding: 8 cores = 4 batch-quarters x 2 LSTM directions (as v1).

Key changes vs v1:
  - fp16 matmul operands everywhere (1 cyc/col vs fp32r ~3).
  - Phase 2 is weights-stationary: gates land TRANSPOSED [units, batch]
    in PSUM, so elementwise uses all 128 lanes and h' needs no per-step
    PE transpose (its layout is already next step's moving operand).
  - Masking is folded into the gate pre-activations: phase 1 adds
    30*(m-1) to z via a K=2 matmul row, so sigmoid/tanh saturate to
    0/-1 on masked steps. Equivalent to reference retention semantics
    because masks are monotone (fwd: suffix masked; bwd: prefix masked).
  - z kept resident in SBUF as fp16; injected into PSUM via one
    identity matmul per step (no phase-2 DMA at all).

Gate chunk order (unit-chunks of 128 on the partition axis):
  chunks 0-3 = g, 4-7 = i, 8-11 = f, 12-15 = o
so tanh(g) can start earliest and sigma(i,f) = one [128,64]-wide
activation; sigma(o) is last and only feeds the final h-mul.
"""

import os

import numpy as np

B, S, H, HL, E, L = 32, 256, 1024, 512, 256, 5
G = 4 * HL            # 2048 gate width
NCORES = 8
BC = 8                # batch items per core
TOK = BC * S          # tokens per core
EPS = 1e-5
P = 128
NEG = 30.0            # mask kill bias

_CACHE = {}
LAST_RUN = {}

# chunk order on the gate axis: [i, g, f, o] x 4 unit-subchunks
_GATE_OF_CHUNK = [0, 0, 0, 0, 2, 2, 2, 2, 1, 1, 1, 1, 3, 3, 3, 3]


def _chunk_perm():
    """perm[c*128+p] = original gate index for chunk c, unit p.
    Torch gate order in weights: i(0) f(1) g(2) o(3)."""
    perm = np.zeros(G, dtype=np.int64)
    for c in range(16):
        gate = _GATE_OF_CHUNK[c]
        sub = [0, 1, 2, 3][c % 4]
        u = np.arange(128) + sub * 128
        perm[c * 128:(c + 1) * 128] = gate * HL + u
    return perm


def _build_nc(nsteps=S, phases=(1, 2, 3)):
    import concourse.tile as tile
    import concourse.mybir as mybir
    from concourse import bacc

    dt = mybir.dt
    f32 = dt.float32
    f16 = dt.float16
    AF = mybir.ActivationFunctionType
    ALU = mybir.AluOpType

    nc = bacc.Bacc(
        "TRN2", target_bir_lowering=False, debug=False, num_devices=NCORES
    )

    # ---------------- I/O ----------------
    xT = nc.dram_tensor("xT", [H, TOK], f16, kind="ExternalInput").ap()
    W1s = nc.dram_tensor("W1s", [BC, H, H], f16, kind="ExternalInput").ap()
    W2s = nc.dram_tensor("W2s", [BC, H, H], f16, kind="ExternalInput").ap()
    # rows 0..3 are b1, ln_g, ln_b, b2; col = item*8 + feat_chunk
    bcols_d = nc.dram_tensor(
        "bcols", [4, P, BC * 8], f32, kind="ExternalInput"
    ).ap()
    # Wih stationary tiles: [k, c, feat128, unit128] (lhsT per tile)
    WihS = nc.dram_tensor("WihS", [8, 16, P, P], f16, kind="ExternalInput").ap()
    # bias+mask: stationary [2, G] rows (b, NEG*ones); moving [2, TOK]
    # rows (ones, m-1)
    biasS = nc.dram_tensor("biasS", [2, G], f16, kind="ExternalInput").ap()
    mrow = nc.dram_tensor("mrow", [2, TOK], f16, kind="ExternalInput").ap()
    WhhS = nc.dram_tensor("WhhS", [4, 16, P, P], f16, kind="ExternalInput").ap()
    WpT = nc.dram_tensor("WpT", [P, 4, E], f16, kind="ExternalInput").ap()
    I128 = nc.dram_tensor("I128", [P, P], f16, kind="ExternalInput").ap()
    OnesP = nc.dram_tensor("OnesP", [P, P], f16, kind="ExternalInput").ap()
    partial = nc.dram_tensor(
        "partial", [TOK, E], f32, kind="ExternalOutput"
    ).ap()

    with tile.TileContext(nc) as tc:
        with tc.tile_pool(name="persist", bufs=1) as persist:
            bcols = persist.tile([P, 4, BC * 8], f32)
            nc.sync.dma_start(out=bcols, in_=bcols_d.rearrange("s p c -> p s c"))
            i128_sb = persist.tile([P, P], f16)
            nc.sync.dma_start(out=i128_sb, in_=I128)
            onesp = persist.tile([P, P], f16)
            nc.sync.dma_start(out=onesp, in_=OnesP)
            eps_sb = persist.tile([P, 1], f32)
            nc.vector.memset(eps_sb, EPS)

            # z resident in SBUF: [128, chunk, token] fp16
            zT = persist.tile([P, 16, TOK], f16)
            # lstm hidden history, unit-major: [128, k, token] fp16
            ysT = persist.tile([P, 4, TOK], f16)

            # ================= PHASE 1 =================
            with (
                tc.tile_pool(name="p1wih", bufs=1) as p1wih,
                tc.tile_pool(name="p1w", bufs=5) as p1w,
                tc.tile_pool(name="p1misc", bufs=1) as p1misc,
                tc.tile_pool(name="p1x", bufs=2) as p1x,
                tc.tile_pool(name="p1a", bufs=2) as p1a,
                tc.tile_pool(name="p1h2", bufs=1) as p1h2,
                tc.tile_pool(name="p1r", bufs=2) as p1r,
                tc.tile_pool(name="psA", bufs=3, space="PSUM") as psA,
                tc.tile_pool(name="psS", bufs=2, space="PSUM") as psS,
                tc.tile_pool(name="psZ", bufs=2, space="PSUM") as psZ,
            ):
                # Wih stationary tiles in SBUF: [128, k, c, 128]
                # (DMA emitted later, at i==1, so it doesn't block the
                # first items' xi/wb loads in the DMA queues)
                wih_sb = p1wih.tile([P, 8, 16, P], f16)

                mrow_sb = p1misc.tile([2, TOK], f16)
                nc.sync.dma_start(out=mrow_sb, in_=mrow)
                biasS_sb = p1misc.tile([2, G], f16)
                nc.sync.dma_start(out=biasS_sb, in_=biasS)
                # h2 quad buffers: [128, featchunk, item-in-quad, S]
                h2q = [
                    p1h2.tile([P, 8, 4, S], f16, name=f"h2q{q}")
                    for q in range(2)
                ]

                def emit_h1(i):
                    """xi DMA + h1 matmuls + inline Square/stat-sums."""
                    xi = p1x.tile([P, 8, S], f16, tag="xi", name=f"xi{i}")
                    nc.sync.dma_start(
                        out=xi,
                        in_=xT[:, i * S:(i + 1) * S].rearrange(
                            "(k p) t -> p k t", p=P
                        ),
                    )
                    a0 = p1a.tile([P, 8, S], f16, tag="a0", name=f"a0_{i}")
                    sps0 = psS.tile([P, S], f32, tag="sps0", bufs=1,
                                    name=f"sps0_{i}")
                    sps1 = psS.tile([P, S], f32, tag="sps1", bufs=1,
                                    name=f"sps1_{i}")
                    for q4 in range(4):
                        wb = p1w.tile([P, 8, 256], f16, tag="w",
                                      name=f"w1b{i}_{q4}")
                        nc.sync.dma_start(
                            out=wb,
                            in_=W1s[i, :, q4 * 256:(q4 + 1) * 256].rearrange(
                                "(k p) m -> p k m", p=P
                            ),
                        )
                        for mm in range(2):
                            m = q4 * 2 + mm
                            ps = psA.tile([P, S], f32, tag="mm",
                                          name=f"ps1_{i}_{m}")
                            for k in range(8):
                                nc.tensor.matmul(
                                    ps, wb[:, k, mm * P:(mm + 1) * P],
                                    xi[:, k, :],
                                    start=(k == 0), stop=(k == 7),
                                )
                            nc.scalar.activation(
                                out=a0[:, m, :], in_=ps, func=AF.Identity,
                                bias=bcols[:, 0, i * 8 + m: i * 8 + m + 1],
                            )
                            sq = p1a.tile([P, S], f16, tag="sq",
                                          name=f"sq{i}_{m}")
                            nc.scalar.activation(
                                out=sq, in_=a0[:, m, :], func=AF.Square,
                            )
                            nc.tensor.matmul(
                                sps0, onesp, a0[:, m, :],
                                start=(m == 0), stop=(m == 7),
                                skip_group_check=True,
                            )
                            nc.tensor.matmul(
                                sps1, onesp, sq,
                                start=(m == 0), stop=(m == 7),
                                skip_group_check=True,
                            )
                    mrB = p1r.tile([P, 2, S], f32, tag="mrB",
                                   name=f"mrB{i}")
                    nc.scalar.activation(
                        out=mrB[:, 0, :], in_=sps0,
                        func=AF.Identity, scale=1.0 / H,
                    )
                    nc.scalar.activation(
                        out=mrB[:, 1, :], in_=sps1,
                        func=AF.Identity, scale=1.0 / H,
                    )
                    scr = p1r.tile([P, S], f32, tag="scr", name=f"scr{i}")
                    nc.vector.tensor_mul(scr, mrB[:, 0, :], mrB[:, 0, :])
                    nc.vector.tensor_sub(scr, mrB[:, 1, :], scr)
                    # rstd = 1/sqrt(|var| + eps); var >= 0 so same as
                    # rsqrt, and this func shares its act table with
                    # identity/square/relu (no ACT_TABLE_LOAD swaps)
                    nc.scalar.activation(out=mrB[:, 1, :], in_=scr,
                                         func=AF.Abs_reciprocal_sqrt,
                                         bias=eps_sb)
                    return a0, mrB

                def emit_rest(i, a0, mrB):
                    """LN apply + h2 for item i."""
                    a1 = p1a.tile([P, 8, S], f16, tag="a1", name=f"a1_{i}")
                    for m in range(8):
                        nc.vector.tensor_sub(
                            a1[:, m, :], a0[:, m, :], mrB[:, 0, :]
                        )
                        nc.vector.tensor_mul(
                            a1[:, m, :], a1[:, m, :], mrB[:, 1, :]
                        )
                        nc.vector.tensor_scalar(
                            out=a1[:, m, :], in0=a1[:, m, :],
                            scalar1=bcols[:, 1, i * 8 + m: i * 8 + m + 1],
                            scalar2=bcols[:, 2, i * 8 + m: i * 8 + m + 1],
                            op0=ALU.mult, op1=ALU.add,
                        )
                        nc.scalar.activation(
                            out=a1[:, m, :], in_=a1[:, m, :], func=AF.Relu,
                        )

                    q, iq = i // 4, i % 4
                    for q4 in range(4):
                        wb = p1w.tile([P, 8, 256], f16, tag="w",
                                      name=f"w2b{i}_{q4}")
                        nc.sync.dma_start(
                            out=wb,
                            in_=W2s[i, :, q4 * 256:(q4 + 1) * 256].rearrange(
                                "(k p) m -> p k m", p=P
                            ),
                        )
                        for mm in range(2):
                            m = q4 * 2 + mm
                            ps = psA.tile([P, S], f32, tag="mm",
                                          name=f"ps2_{i}_{m}")
                            for k in range(8):
                                nc.tensor.matmul(
                                    ps, wb[:, k, mm * P:(mm + 1) * P],
                                    a1[:, k, :],
                                    start=(k == 0), stop=(k == 7),
                                )
                            nc.scalar.activation(
                                out=h2q[q][:, m, iq, :], in_=ps,
                                func=AF.Identity,
                                bias=bcols[:, 3, i * 8 + m: i * 8 + m + 1],
                            )

                def emit_z(q):
                    for c in range(16):
                        for th in range(2):  # 512-token halves of quad
                            zp = psZ.tile([P, 512], f32, tag="zp",
                                          name=f"zp{q}_{c}_{th}")
                            rhs = h2q[q].rearrange("p m i t -> p m (i t)")
                            for k in range(8):
                                nc.tensor.matmul(
                                    zp, wih_sb[:, k, c, :],
                                    rhs[:, k, th * 512:(th + 1) * 512],
                                    start=(k == 0), stop=False,
                                )
                            tsl = slice(q * 1024 + th * 512,
                                        q * 1024 + (th + 1) * 512)
                            nc.tensor.matmul(
                                zp, biasS_sb[:, c * P:(c + 1) * P],
                                mrow_sb[:, tsl],
                                start=False, stop=True,
                            )
                            nc.scalar.activation(
                                out=zT[:, c, tsl], in_=zp,
                                func=AF.Identity,
                            )

                # software pipeline: h1 of item i+1 is emitted before the
                # LN/h2 of item i so the PE FIFO never drains during the
                # LN latency chain
                if 1 in phases:
                    pending = emit_h1(0)
                    for i in range(BC):
                        nxt = emit_h1(i + 1) if i + 1 < BC else None
                        if i == 1:
                            nc.sync.dma_start(
                                out=wih_sb,
                                in_=WihS.rearrange("k c p u -> p k c u"),
                            )
                        emit_rest(i, *pending)
                        pending = nxt
                        if i % 4 == 3:
                            emit_z(i // 4)

            # ================= PHASE 2 =================
            with (
                tc.tile_pool(name="p2whh", bufs=1) as p2whh,
                tc.tile_pool(name="p2s", bufs=1) as p2s,
                tc.tile_pool(name="p2t", bufs=2) as p2t,
                tc.tile_pool(name="psG", bufs=2, space="PSUM") as psG,
            ):
                whh_sb = p2whh.tile([P, 4, 16, P], f16)
                # finer-grained DMAs: step 1's first matmuls unblock
                # after one eighth of the transfer
                for kq in range(4):
                    for ch in range(2):
                        nc.sync.dma_start(
                            out=whh_sb[:, kq, ch * 8:(ch + 1) * 8, :],
                            in_=WhhS[kq, ch * 8:(ch + 1) * 8].rearrange(
                                "c p u -> p c u"),
                        )
                c_st = p2s.tile([P, 4, BC], f32)
                nc.vector.memset(c_st.rearrange("p a b -> p (a b)"), 0.0)
                # zT tokens are item-major (i*S + t); view for per-step
                # strided reads of all 8 items at time s
                zT4 = zT.rearrange("p c (i t) -> p c i t", i=BC)

                for s in range(nsteps if 2 in phases else 0):
                    # one PSUM tile per gate -> independent dep tracking,
                    # so each gate's activation fires right after its own
                    # accumulation group stops (not after the whole burst)
                    gps = [
                        psG.tile([P, 4, BC], f32, tag=f"g{g}",
                                 name=f"gp{g}_{s}")
                        for g in range(4)
                    ]
                    bsl = slice(s * BC, (s + 1) * BC)
                    psl = slice((s - 1) * BC, s * BC)
                    # gate order: 0 i, 1 g, 2 f, 3 o (chunks 4g..4g+3)
                    for g in range(4):
                        csl = slice(4 * g, 4 * g + 4)
                        nc.tensor.matmul(
                            gps[g].rearrange("p c b -> p (c b)"),
                            i128_sb,
                            zT4[:, csl, :, s].rearrange("p c i -> p (c i)"),
                            start=True, stop=(s == 0),
                        )
                        if s > 0:
                            for cc in range(4):
                                c = 4 * g + cc
                                for k in range(4):
                                    nc.tensor.matmul(
                                        gps[g][:, cc, :], whh_sb[:, k, c, :],
                                        ysT[:, k, psl],
                                        start=False,
                                        stop=(cc == 3 and k == 3),
                                        skip_group_check=True,
                                    )
                    sig = p2t.tile([P, 16, BC], f32, tag="sig")
                    nc.scalar.activation(
                        out=sig[:, 0:4, :].rearrange("p c b -> p (c b)"),
                        in_=gps[0].rearrange("p c b -> p (c b)"),
                        func=AF.Sigmoid,
                    )
                    nc.scalar.activation(
                        out=sig[:, 4:8, :].rearrange("p c b -> p (c b)"),
                        in_=gps[1].rearrange("p c b -> p (c b)"),
                        func=AF.Tanh,
                    )
                    # t2 = sigma(i) * tanh(g) can run while f/o matmuls go
                    t2 = p2t.tile([P, 4, BC], f32, tag="t2")
                    nc.vector.tensor_mul(t2, sig[:, 0:4, :], sig[:, 4:8, :])
                    nc.scalar.activation(
                        out=sig[:, 8:12, :].rearrange("p c b -> p (c b)"),
                        in_=gps[2].rearrange("p c b -> p (c b)"),
                        func=AF.Sigmoid,
                    )
                    nc.scalar.activation(
                        out=sig[:, 12:16, :].rearrange("p c b -> p (c b)"),
                        in_=gps[3].rearrange("p c b -> p (c b)"),
                        func=AF.Sigmoid,
                    )
                    t1 = p2t.tile([P, 4, BC], f32, tag="t1")
                    nc.vector.tensor_mul(t1, sig[:, 8:12, :], c_st)
                    nc.vector.tensor_add(c_st, t1, t2)
                    tc3 = p2t.tile([P, 4, BC], f32, tag="tc")
                    nc.scalar.activation(
                        out=tc3.rearrange("p a b -> p (a b)"),
                        in_=c_st.rearrange("p a b -> p (a b)"),
                        func=AF.Tanh,
                    )
                    nc.vector.tensor_mul(
                        ysT[:, :, bsl], sig[:, 12:16, :], tc3,
                    )

            # ================= PHASE 3 =================
            with (
                tc.tile_pool(name="p3", bufs=4) as p3,
                tc.tile_pool(name="p3w", bufs=1) as p3w,
                tc.tile_pool(name="psP", bufs=4, space="PSUM") as psP,
            ):
                wp_sb = p3w.tile([P, 4, E], f16)
                nc.sync.dma_start(out=wp_sb, in_=WpT)
                for mt in range(TOK // P if 3 in phases else 0):
                    pp = psP.tile([P, E], f32, tag="pp")
                    for k in range(4):
                        nc.tensor.matmul(
                            pp, ysT[:, k, mt * P:(mt + 1) * P],
                            wp_sb[:, k, :],
                            start=(k == 0), stop=(k == 3),
                        )
                    ot = p3.tile([P, E], f32, tag="ot")
                    nc.scalar.activation(out=ot, in_=pp, func=AF.Identity)
                    nc.sync.dma_start(
                        out=partial[mt * P:(mt + 1) * P, :], in_=ot
                    )

    nc.finalize()
    return nc


def _prep_core_inputs(core, perm, seq, am, li, W1, b1, ln_g, ln_b, W2, b2,
                      Wih, Whh, bvec, Wp):
    q = core % 4
    bwd = core >= 4
    items = perm[q * BC:(q + 1) * BC]
    cperm = _chunk_perm()

    x = seq[items]                          # [8, S, H]
    mm = am[items].astype(np.float32)       # [8, S]
    if bwd:
        x = x[:, ::-1, :]
        mm = mm[:, ::-1]
    xT = np.ascontiguousarray(
        x.transpose(2, 0, 1).reshape(H, TOK), dtype=np.float16
    )
    langs = li[items]
    W1s = np.ascontiguousarray(W1[langs], dtype=np.float16)
    W2s = np.ascontiguousarray(W2[langs], dtype=np.float16)

    def cols(v):                            # [L,1024] -> [128, item*8+m]
        vv = v[langs]
        return vv.reshape(BC, 8, P).transpose(2, 0, 1).reshape(P, BC * 8)

    bcols = np.ascontiguousarray(
        np.stack([cols(b1), cols(ln_g), cols(ln_b), cols(b2)], axis=0),
        dtype=np.float32,
    )

    # Wih stationary tiles [k, c, feat128, unit128]: lhsT = Wih.T chunk
    WihP = Wih[cperm, :]                    # [G, H] permuted gate rows
    WihS = np.ascontiguousarray(
        WihP.reshape(16, P, 8, P).transpose(2, 0, 3, 1), dtype=np.float16
    )  # [k, c, feat, unit]
    biasS = np.empty((2, G), dtype=np.float16)
    biasS[0] = bvec[cperm]
    biasS[1] = NEG
    mrow = np.empty((2, TOK), dtype=np.float16)
    mrow[0] = 1.0
    mrow[1] = (mm - 1.0).reshape(TOK)

    WhhP = Whh[cperm, :]                    # [G, HL]
    WhhS = np.ascontiguousarray(
        WhhP.reshape(16, P, 4, P).transpose(2, 0, 3, 1), dtype=np.float16
    )  # [k, c, feat, unit]

    d0 = HL if bwd else 0
    WpT = np.ascontiguousarray(
        Wp[:, d0:d0 + HL].T.reshape(4, P, E).transpose(1, 0, 2),
        dtype=np.float16,
    )  # [p, k, e]

    return {
        "xT": xT, "W1s": W1s, "W2s": W2s, "bcols": bcols,
        "WihS": WihS, "biasS": biasS, "mrow": mrow, "WhhS": WhhS,
        "WpT": WpT, "I128": np.eye(P, dtype=np.float16),
        "OnesP": np.ones((P, P), dtype=np.float16),
    }


def kernel(sequence_output, attention_mask, language_ids, W1, b1, ln_g, ln_b,
           W2, b2, Wih_f, Whh_f, b_f, Wih_b, Whh_b, b_b, Wp, bp):
    from concourse.bass_utils import run_bass_kernel_spmd

    seq = np.asarray(sequence_output, dtype=np.float32)
    am = np.asarray(attention_mask)
    li = np.asarray(language_ids).astype(np.int64)

    key = "nc2"
    if key not in _CACHE:
        _CACHE[key] = _build_nc()
    nc = _CACHE[key]

    perm = np.argsort(li, kind="stable")
    in_maps = []
    for core in range(NCORES):
        bwd = core >= 4
        in_maps.append(
            _prep_core_inputs(
                core, perm, seq, am, li,
                np.asarray(W1, np.float32), np.asarray(b1, np.float32),
                np.asarray(ln_g, np.float32), np.asarray(ln_b, np.float32),
                np.asarray(W2, np.float32), np.asarray(b2, np.float32),
                np.asarray(Wih_b if bwd else Wih_f, np.float32),
                np.asarray(Whh_b if bwd else Whh_f, np.float32),
                np.asarray(b_b if bwd else b_f, np.float32),
                np.asarray(Wp, np.float32),
            )
        )

    trace = bool(os.environ.get("KERNEL_TRACE"))
    res = run_bass_kernel_spmd(
        nc, in_maps, core_ids=list(range(NCORES)), trace=trace
    )
    LAST_RUN["exec_time_ns"] = res.exec_time_ns
    LAST_RUN["profile_json"] = res.profile_json
    # partial rows are ordered (t, b_local): ysT tokens are step-major
    outs = [
        r["partial"].reshape(S, BC, E).transpose(1, 0, 2) for r in res.results
    ]

    out = np.empty((B, S, E), dtype=np.float32)
    bp32 = np.asarray(bp, dtype=np.float32)
    for q in range(4):
        items = perm[q * BC:(q + 1) * BC]
        pf = outs[q]                        # [8, S, E]
        pb = outs[q + 4][:, ::-1, :]        # un-reverse time
        out[items] = pf + pb + bp32
    return out
